# revision 12
# baseline (speedup 1.0000x reference)
"""Bicubic 2x downsample (4,3,768,768) -> (4,3,384,384) plus the broadcast
per-pixel kernel-weights tensor, run SPMD across 8 TRN2 NeuronCores.

Because the scale factor is exactly 2, the fractional sample offsets are
identical for every output pixel, so the 6x6 bicubic weight matrix is one
constant separable outer product wn[p] * wn[q].

Device kernel per core (core r handles output rows 48r..48r+47):
  - input: host-prepared slab [100, 12*772]: for each of the 12 (b,c)
    images, input rows 96r-2..96r+97 (edge-clamped) with 2+2 edge-clamped
    pad columns; partition dim = input row.
  - res: for each image, 6 accumulating fp32 matmuls into a PSUM tile
    [48, 384]: matmul q uses stationary weights wq[q][h][i] = wn[q] *
    wn[h-2i] [100 x 48] and moving operand xpad[:, q : q+767 : 2]
    [100 x 384].  This computes the full separable 6x6 bicubic in one
    accumulation group (vertical via the contraction over rows, horizontal
    via the 6 strided taps).  ACT copies each PSUM tile into an SBUF result
    slab; one DMA stores it at the end.
  - kernel-weights output: DVE replicates the 36-value pattern into an SBUF
    tile [128, 2*5184]; two DMAs write it out (2 batch elements each).

Raw Bass (no Tile): the walrus build in this container allows at most one
sync-wait command per instruction, which Tile's scheduler (multi-wait drain
and matmuls) violates.  Here every cross-engine dependency is a standalone
wait_ge instruction and every DMA gets its own semaphore.
"""

from contextlib import ExitStack

import numpy as np

import concourse.bass as bass
import concourse.mybir as mybir
from concourse.bass_utils import run_bass_kernel_spmd

B, C, H, W = 4, 3, 768, 768
OH = OW = 384
NIMG = B * C                 # 12 images
NCORES = 8
RPC = OH // NCORES           # 48 output rows per core
IN_ROWS = 2 * RPC + 4        # 100 input rows per core
PT = 6                       # taps per axis
XW = W + 4                   # padded image width (772)
KCOLS = RPC * OW * 36 // 128  # 5184: per-partition cols of one b's kernel slab
CONST_COLS = 36 + PT * RPC   # pattern seed + weight matrices
NPS = 7                      # PSUM tiles cycled across images

# matmul dtype mode: "f32" exact (4 cyc/row) or "f32r" (1 cyc/row)
MM_MODE = "f32"

CHUNKS = [0, 1, 3, 6, NIMG]  # input-load chunk boundaries (in images)


def _cubic(x):
    a = np.abs(x)
    a2 = a * a
    a3 = a2 * a
    return np.where(
        a <= 1.0,
        1.5 * a3 - 2.5 * a2 + 1.0,
        np.where(a <= 2.0, -0.5 * a3 + 2.5 * a2 - 4.0 * a + 2.0, 0.0),
    )


def _tap_weights():
    # offsets before clipping are constant for exact 2x: off[k] = 2.5 - k,
    # scaled by out/in = 0.5 before the cubic.
    k = np.arange(PT, dtype=np.float64)
    w = _cubic((2.5 - k) * 0.5)
    return w / w.sum()


_WN = _tap_weights()                               # normalized taps [6]
WVT = np.zeros((IN_ROWS, RPC), np.float64)         # vertical weights, lhsT
for _i in range(RPC):
    WVT[2 * _i : 2 * _i + PT, _i] = _WN
# 6 stationary matrices [100, 6*48]: wq[:, q*48:(q+1)*48] = wn[q] * WVT
WQ = np.concatenate([(_WN[q] * WVT) for q in range(PT)], axis=1).astype(np.float32)
WMAT36 = np.outer(_WN, _WN).reshape(36).astype(np.float32)

CONST = np.zeros((128, CONST_COLS), np.float32)
CONST[:, 0:36] = WMAT36[None, :]
CONST[0:IN_ROWS, 36:] = WQ

_CACHE = {}


def build_bass(mode=MM_MODE):
    key = ("nc", mode)
    if key in _CACHE:
        return _CACHE[key]
    nc = bass.Bass("TRN2", target_bir_lowering=False, debug=False, num_devices=NCORES)
    f32 = mybir.dt.float32
    x_d = nc.dram_tensor("x_in", [IN_ROWS, NIMG * XW], f32, kind="ExternalInput").ap()
    c_d = nc.dram_tensor("const_in", [128, CONST_COLS], f32, kind="ExternalInput").ap()
    res_d = nc.dram_tensor("res_out", [NIMG, RPC, OW], f32, kind="ExternalOutput").ap()
    ker_d = nc.dram_tensor("ker_out", [B, 128, KCOLS], f32, kind="ExternalOutput").ap()

    def mmcast(ap):
        return ap.bitcast(mybir.dt.float32r) if mode == "f32r" else ap

    with ExitStack() as ctx:
        const_sb = ctx.enter_context(
            nc.sbuf_tensor("const_sb", [128, CONST_COLS], f32)
        ).ap()
        xt = ctx.enter_context(nc.sbuf_tensor("xt", [IN_ROWS, NIMG * XW], f32)).ap()
        patbig = ctx.enter_context(
            nc.sbuf_tensor("patbig", [128, 2 * KCOLS], f32)
        ).ap()
        rt = ctx.enter_context(nc.sbuf_tensor("rt", [RPC, NIMG * OW], f32)).ap()
        pss = [
            ctx.enter_context(nc.psum_tensor(f"ps{i}", [RPC, OW], f32)).ap()
            for i in range(NPS)
        ]

        s_const = ctx.enter_context(nc.semaphore("s_const"))
        s_chunk = [
            ctx.enter_context(nc.semaphore(f"s_chunk{k}")) for k in range(4)
        ]
        s_ker = [ctx.enter_context(nc.semaphore(f"s_ker{g}")) for g in range(2)]
        s_res = ctx.enter_context(nc.semaphore("s_res"))
        psem = ctx.enter_context(nc.semaphore("psem"))
        asem = ctx.enter_context(nc.semaphore("asem"))
        vsem = ctx.enter_context(nc.semaphore("vsem"))
        fsem = ctx.enter_context(nc.semaphore("fsem"))

        wq_sb = const_sb[0:IN_ROWS, 36:CONST_COLS]
        blk = ctx.enter_context(nc.Block())

        @blk.sync
        def _(sync):
            sync.dma_start(const_sb, c_d).then_inc(s_const, 16)
            for k in range(4):
                a, b = CHUNKS[k], CHUNKS[k + 1]
                sync.dma_start(
                    xt[:, a * XW : b * XW], x_d[:, a * XW : b * XW]
                ).then_inc(s_chunk[k], 16)
            sync.wait_ge(vsem, 1)
            for g in range(2):
                sync.dma_start(
                    ker_d[2 * g : 2 * g + 2].rearrange("b p f -> p b f"),
                    patbig.rearrange("p (b f) -> p b f", b=2),
                ).then_inc(s_ker[g], 16)
            sync.wait_ge(asem, NIMG)
            sync.dma_start(
                res_d.rearrange("g r c -> r g c"),
                rt.rearrange("r (g c) -> r g c", g=NIMG),
            ).then_inc(s_res, 16)
            sync.wait_ge(s_ker[0], 16)
            sync.wait_ge(s_ker[1], 16)
            sync.wait_ge(s_res, 16)

        @blk.tensor
        def _(tensor):
            tensor.wait_ge(s_const, 16)
            for img in range(NIMG):
                if img in CHUNKS:
                    tensor.wait_ge(s_chunk[CHUNKS.index(img)], 16)
                if img >= NPS:
                    # psum slot reuse: copy of image img-NPS must be done
                    tensor.wait_ge(asem, img - NPS + 1)
                x0 = img * XW
                ps = pss[img % NPS]
                for q in range(PT):
                    ins = nc.tensor.matmul(
                        ps,
                        mmcast(wq_sb[:, q * RPC : (q + 1) * RPC]),
                        mmcast(xt[:, x0 + q : x0 + q + 767 : 2]),
                        start=(q == 0),
                        stop=(q == PT - 1),
                    )
                ins.then_inc(psem, 1)

        @blk.scalar
        def _(scalar):
            for img in range(NIMG):
                scalar.wait_ge(psem, img + 1)
                nc.scalar.copy(
                    rt[:, img * OW : (img + 1) * OW], pss[img % NPS]
                ).then_inc(asem, 1)

        @blk.vector
        def _(vector):
            vector.wait_ge(s_const, 16)
            # doubling fill; DVE ops read what the previous one wrote, so each
            # step is gated on fsem (same-engine RAW still needs a sem)
            nc.vector.tensor_copy(patbig[:, 0:36], const_sb[:, 0:36]).then_inc(
                fsem, 1
            )
            filled = 36
            step = 1
            while filled < 2 * KCOLS:
                n = min(filled, 2 * KCOLS - filled)
                vector.wait_ge(fsem, step)
                ins = nc.vector.tensor_copy(
                    patbig[:, filled : filled + n], patbig[:, 0:n]
                )
                filled += n
                step += 1
                if filled < 2 * KCOLS:
                    ins.then_inc(fsem, 1)
                else:
                    ins.then_inc(vsem, 1)

    _CACHE[key] = nc
    return nc


def make_in_maps(x):
    xf = np.ascontiguousarray(np.asarray(x, dtype=np.float32)).reshape(NIMG, H, W)
    cols = np.clip(np.arange(-2, W + 2), 0, W - 1)
    in_maps = []
    for r in range(NCORES):
        rows = np.clip(np.arange(96 * r - 2, 96 * r + 98), 0, H - 1)
        slab = xf[:, rows[:, None], cols[None, :]]        # [12, 100, 772]
        slab = slab.transpose(1, 0, 2).reshape(IN_ROWS, NIMG * XW)
        in_maps.append(
            {
                "x_in": np.ascontiguousarray(slab),
                "const_in": CONST,
            }
        )
    return in_maps


def gather_outputs(results):
    res = np.empty((NIMG, OH, OW), np.float32)
    ker = np.empty((B, 1, OH, OW, 36), np.float32)
    for r, m in enumerate(results):
        res[:, RPC * r : RPC * (r + 1), :] = m["res_out"]
        ker[:, 0, RPC * r : RPC * (r + 1)] = m["ker_out"].reshape(B, RPC, OW, 36)
    return res.reshape(B, C, OH, OW), ker


def kernel(input, out_h, out_w):
    assert int(out_h) == OH and int(out_w) == OW
    x = np.asarray(input, dtype=np.float32)
    assert x.shape == (B, C, H, W)
    nc = build_bass()
    out = run_bass_kernel_spmd(nc, make_in_maps(x), core_ids=list(range(NCORES)))
    return gather_outputs(out.results)


# revision 15
# speedup vs baseline: 1.0308x; 1.0308x over previous
"""Bicubic 2x downsample (4,3,768,768) -> (4,3,384,384) plus the broadcast
per-pixel kernel-weights tensor, run SPMD across 8 TRN2 NeuronCores.

Because the scale factor is exactly 2, the fractional sample offsets are
identical for every output pixel, so the 6x6 bicubic weight matrix is one
constant separable outer product wn[p] * wn[q].

Device kernel per core (core r handles output rows 48r..48r+47):
  - input: host-prepared slab [100, 12*772]: for each of the 12 (b,c)
    images, input rows 96r-2..96r+97 (edge-clamped) with 2+2 edge-clamped
    pad columns, stored as [386 even cols | 386 odd cols] so every tap's
    moving operand is a contiguous slice; partition dim = input row.
  - res: for each image, 6 accumulating fp32 matmuls into a PSUM tile
    [48, 384]: matmul q uses stationary weights wq[q][h][i] = wn[q] *
    wn[h-2i] [100 x 48] and moving operand plane[q%2][:, q//2 : q//2+384]
    [100 x 384].  This computes the full separable 6x6 bicubic in one
    accumulation group (vertical via the contraction over rows, horizontal
    via the 6 strided taps).  ACT copies each PSUM tile into an SBUF result
    slab; one DMA stores it at the end.
  - kernel-weights output: DVE replicates the 36-value pattern into an SBUF
    tile [128, 2*5184]; two DMAs write it out (2 batch elements each).

Raw Bass (no Tile): the walrus build in this container allows at most one
sync-wait command per instruction, which Tile's scheduler (multi-wait drain
and matmuls) violates.  Here every cross-engine dependency is a standalone
wait_ge instruction and every DMA gets its own semaphore.
"""

from contextlib import ExitStack

import numpy as np

import concourse.bass as bass
import concourse.mybir as mybir
from concourse.bass_utils import run_bass_kernel_spmd

B, C, H, W = 4, 3, 768, 768
OH = OW = 384
NIMG = B * C                 # 12 images
NCORES = 8
RPC = OH // NCORES           # 48 output rows per core
IN_ROWS = 2 * RPC + 4        # 100 input rows per core
PT = 6                       # taps per axis
XW = W + 4                   # padded image width (772)
KCOLS = RPC * OW * 36 // 128  # 5184: per-partition cols of one b's kernel slab
CONST_COLS = 36 + PT * RPC   # pattern seed + weight matrices
NPS = 7                      # PSUM tiles cycled across images

# matmul dtype mode: "f32" exact (4 cyc/row) or "f32r" (1 cyc/row)
MM_MODE = "f32"

CHUNKS = [0, 1, 3, 6, NIMG]  # input-load chunk boundaries (in images)


def _cubic(x):
    a = np.abs(x)
    a2 = a * a
    a3 = a2 * a
    return np.where(
        a <= 1.0,
        1.5 * a3 - 2.5 * a2 + 1.0,
        np.where(a <= 2.0, -0.5 * a3 + 2.5 * a2 - 4.0 * a + 2.0, 0.0),
    )


def _tap_weights():
    # offsets before clipping are constant for exact 2x: off[k] = 2.5 - k,
    # scaled by out/in = 0.5 before the cubic.
    k = np.arange(PT, dtype=np.float64)
    w = _cubic((2.5 - k) * 0.5)
    return w / w.sum()


_WN = _tap_weights()                               # normalized taps [6]
WVT = np.zeros((IN_ROWS, RPC), np.float64)         # vertical weights, lhsT
for _i in range(RPC):
    WVT[2 * _i : 2 * _i + PT, _i] = _WN
# 6 stationary matrices [100, 6*48]: wq[:, q*48:(q+1)*48] = wn[q] * WVT
WQ = np.concatenate([(_WN[q] * WVT) for q in range(PT)], axis=1).astype(np.float32)
WMAT36 = np.outer(_WN, _WN).reshape(36).astype(np.float32)

CONST = np.zeros((128, CONST_COLS), np.float32)
CONST[:, 0:36] = WMAT36[None, :]
CONST[0:IN_ROWS, 36:] = WQ

_CACHE = {}


def build_bass(mode=MM_MODE):
    key = ("nc", mode)
    if key in _CACHE:
        return _CACHE[key]
    nc = bass.Bass("TRN2", target_bir_lowering=False, debug=False, num_devices=NCORES)
    f32 = mybir.dt.float32
    x_d = nc.dram_tensor("x_in", [IN_ROWS, NIMG * XW], f32, kind="ExternalInput").ap()
    c_d = nc.dram_tensor("const_in", [128, CONST_COLS], f32, kind="ExternalInput").ap()
    res_d = nc.dram_tensor("res_out", [NIMG, RPC, OW], f32, kind="ExternalOutput").ap()
    ker_d = nc.dram_tensor("ker_out", [B, 128, KCOLS], f32, kind="ExternalOutput").ap()

    def mmcast(ap):
        return ap.bitcast(mybir.dt.float32r) if mode == "f32r" else ap

    with ExitStack() as ctx:
        const_sb = ctx.enter_context(
            nc.sbuf_tensor("const_sb", [128, CONST_COLS], f32)
        ).ap()
        xt = ctx.enter_context(nc.sbuf_tensor("xt", [IN_ROWS, NIMG * XW], f32)).ap()
        patbig = ctx.enter_context(
            nc.sbuf_tensor("patbig", [128, 2 * KCOLS], f32)
        ).ap()
        rt = ctx.enter_context(nc.sbuf_tensor("rt", [RPC, NIMG * OW], f32)).ap()
        pss = [
            ctx.enter_context(nc.psum_tensor(f"ps{i}", [RPC, OW], f32)).ap()
            for i in range(NPS)
        ]

        s_const = ctx.enter_context(nc.semaphore("s_const"))
        s_chunk = [
            ctx.enter_context(nc.semaphore(f"s_chunk{k}")) for k in range(4)
        ]
        s_ker = [ctx.enter_context(nc.semaphore(f"s_ker{g}")) for g in range(2)]
        s_res = ctx.enter_context(nc.semaphore("s_res"))
        psem = ctx.enter_context(nc.semaphore("psem"))
        asem = ctx.enter_context(nc.semaphore("asem"))
        vsem = ctx.enter_context(nc.semaphore("vsem"))
        fsem = ctx.enter_context(nc.semaphore("fsem"))

        wq_sb = const_sb[0:IN_ROWS, 36:CONST_COLS]
        blk = ctx.enter_context(nc.Block())

        @blk.sync
        def _(sync):
            sync.dma_start(const_sb, c_d).then_inc(s_const, 16)
            for k in range(4):
                a, b = CHUNKS[k], CHUNKS[k + 1]
                sync.dma_start(
                    xt[:, a * XW : b * XW], x_d[:, a * XW : b * XW]
                ).then_inc(s_chunk[k], 16)
            sync.wait_ge(vsem, 1)
            for g in range(2):
                sync.dma_start(
                    ker_d[2 * g : 2 * g + 2].rearrange("b p f -> p b f"),
                    patbig.rearrange("p (b f) -> p b f", b=2),
                ).then_inc(s_ker[g], 16)
            sync.wait_ge(asem, NIMG)
            sync.dma_start(
                res_d.rearrange("g r c -> r g c"),
                rt.rearrange("r (g c) -> r g c", g=NIMG),
            ).then_inc(s_res, 16)
            sync.wait_ge(s_ker[0], 16)
            sync.wait_ge(s_ker[1], 16)
            sync.wait_ge(s_res, 16)

        @blk.tensor
        def _(tensor):
            tensor.wait_ge(s_const, 16)
            for img in range(NIMG):
                if img in CHUNKS:
                    tensor.wait_ge(s_chunk[CHUNKS.index(img)], 16)
                if img >= NPS:
                    # psum slot reuse: copy of image img-NPS must be done
                    tensor.wait_ge(asem, img - NPS + 1)
                x0 = img * XW
                ps = pss[img % NPS]
                for q in range(PT):
                    base = x0 + (q % 2) * (XW // 2) + q // 2
                    ins = nc.tensor.matmul(
                        ps,
                        mmcast(wq_sb[:, q * RPC : (q + 1) * RPC]),
                        mmcast(xt[:, base : base + OW]),
                        start=(q == 0),
                        stop=(q == PT - 1),
                    )
                ins.then_inc(psem, 1)

        @blk.scalar
        def _(scalar):
            for img in range(NIMG):
                scalar.wait_ge(psem, img + 1)
                nc.scalar.copy(
                    rt[:, img * OW : (img + 1) * OW], pss[img % NPS]
                ).then_inc(asem, 1)

        @blk.vector
        def _(vector):
            vector.wait_ge(s_const, 16)
            # doubling fill; DVE ops read what the previous one wrote, so each
            # step is gated on fsem (same-engine RAW still needs a sem)
            nc.vector.tensor_copy(patbig[:, 0:36], const_sb[:, 0:36]).then_inc(
                fsem, 1
            )
            filled = 36
            step = 1
            while filled < 2 * KCOLS:
                n = min(filled, 2 * KCOLS - filled)
                vector.wait_ge(fsem, step)
                ins = nc.vector.tensor_copy(
                    patbig[:, filled : filled + n], patbig[:, 0:n]
                )
                filled += n
                step += 1
                if filled < 2 * KCOLS:
                    ins.then_inc(fsem, 1)
                else:
                    ins.then_inc(vsem, 1)

    _CACHE[key] = nc
    return nc


def make_in_maps(x):
    xf = np.ascontiguousarray(np.asarray(x, dtype=np.float32)).reshape(NIMG, H, W)
    cols = np.clip(np.arange(-2, W + 2), 0, W - 1)
    # even/odd split of the padded columns: tap q reads the contiguous range
    # plane[q%2][:, q//2 : q//2+384]
    cols = np.concatenate([cols[0::2], cols[1::2]])
    in_maps = []
    for r in range(NCORES):
        rows = np.clip(np.arange(96 * r - 2, 96 * r + 98), 0, H - 1)
        slab = xf[:, rows[:, None], cols[None, :]]        # [12, 100, 772]
        slab = slab.transpose(1, 0, 2).reshape(IN_ROWS, NIMG * XW)
        in_maps.append(
            {
                "x_in": np.ascontiguousarray(slab),
                "const_in": CONST,
            }
        )
    return in_maps


def gather_outputs(results):
    res = np.empty((NIMG, OH, OW), np.float32)
    ker = np.empty((B, 1, OH, OW, 36), np.float32)
    for r, m in enumerate(results):
        res[:, RPC * r : RPC * (r + 1), :] = m["res_out"]
        ker[:, 0, RPC * r : RPC * (r + 1)] = m["ker_out"].reshape(B, RPC, OW, 36)
    return res.reshape(B, C, OH, OW), ker


def kernel(input, out_h, out_w):
    assert int(out_h) == OH and int(out_w) == OW
    x = np.asarray(input, dtype=np.float32)
    assert x.shape == (B, C, H, W)
    nc = build_bass()
    out = run_bass_kernel_spmd(nc, make_in_maps(x), core_ids=list(range(NCORES)))
    return gather_outputs(out.results)


# revision 23
# speedup vs baseline: 1.1954x; 1.1597x over previous
"""Bicubic 2x downsample (4,3,768,768) -> (4,3,384,384) plus the broadcast
per-pixel kernel-weights tensor, run SPMD across 8 TRN2 NeuronCores.

Because the scale factor is exactly 2, the fractional sample offsets are
identical for every output pixel, so the 6x6 bicubic weight matrix is one
constant separable outer product wn[p] * wn[q].

Device kernel per core (core r handles output rows 48r..48r+47), all fp32:
  - input: host-prepared slab [100, 12*772]: for each of the 12 (b,c)
    images, input rows 96r-2..96r+97 (edge-clamped) with 2+2 edge-clamped
    pad columns; partition dim = input row.
  - vertical pass (PE): per image TWO matmuls (N=512 and N=260, K=100,
    M=48) with stationary weights wvt[h][i] = wn[h-2i], producing the
    column-padded vertical convolution Vpad [48, 772] in PSUM.  Images are
    stacked in pairs at PSUM partition bases 0 and 64 (two banks per pair).
  - ACT copies the PSUM pair into a padded SBUF tile [128, 772].
  - horizontal pass (DVE): 6 chained scalar_tensor_tensor ops per pair,
    out[j] += wn[q] * Vpad[:, 2j+q], on [112, 384] shapes (both images of
    the pair in one op; rows 48:64 are zeroed garbage).
  - kernel-weights output: DVE replicates the 36-value pattern into an SBUF
    tile [128, 2*5184]; two DMAs write it out (2 batch elements each).

Raw Bass (no Tile): the walrus build in this container allows at most one
sync-wait command per instruction, which Tile's scheduler violates.  Every
cross-engine dependency here is a standalone wait_ge instruction, each DMA
has its own semaphore, and same-engine RAW chains are serialized with a
counter semaphore (the CoreSim race detector requires it).
"""

from contextlib import ExitStack

import numpy as np

import concourse.bass as bass
import concourse.mybir as mybir
from concourse.bass_utils import run_bass_kernel_spmd

B, C, H, W = 4, 3, 768, 768
OH = OW = 384
NIMG = B * C                 # 12 images
NPAIR = NIMG // 2            # 6 image pairs
NCORES = 8
RPC = OH // NCORES           # 48 output rows per core
IN_ROWS = 2 * RPC + 4        # 100 input rows per core
PT = 6                       # taps per axis
XW = W + 4                   # padded image width (772)
NA, NB = 512, XW - 512       # vertical matmul column split (512 + 260)
KCOLS = RPC * OW * 36 // 128  # 5184: per-partition cols of one b's kernel slab
CONST_COLS = 36 + RPC        # pattern seed + vertical weight matrix
NSLOT = 3                    # PSUM pair-slots (2 banks each)

CHUNKS = [0, 1, 3, 6, NIMG]  # input-load chunk boundaries (in images)


def _cubic(x):
    a = np.abs(x)
    a2 = a * a
    a3 = a2 * a
    return np.where(
        a <= 1.0,
        1.5 * a3 - 2.5 * a2 + 1.0,
        np.where(a <= 2.0, -0.5 * a3 + 2.5 * a2 - 4.0 * a + 2.0, 0.0),
    )


def _tap_weights():
    # offsets before clipping are constant for exact 2x: off[k] = 2.5 - k,
    # scaled by out/in = 0.5 before the cubic.
    k = np.arange(PT, dtype=np.float64)
    w = _cubic((2.5 - k) * 0.5)
    return w / w.sum()


_WN = _tap_weights()                               # normalized taps [6]
WVT = np.zeros((IN_ROWS, RPC), np.float32)         # vertical weights, lhsT
for _i in range(RPC):
    WVT[2 * _i : 2 * _i + PT, _i] = _WN
WN32 = _WN.astype(np.float32)
WMAT36 = np.outer(_WN, _WN).reshape(36).astype(np.float32)

CONST = np.zeros((128, CONST_COLS), np.float32)
CONST[:, 0:36] = WMAT36[None, :]
CONST[0:IN_ROWS, 36:] = WVT

_CACHE = {}


def build_bass():
    if "nc" in _CACHE:
        return _CACHE["nc"]
    nc = bass.Bass("TRN2", target_bir_lowering=False, debug=False, num_devices=NCORES)
    f32 = mybir.dt.float32
    x_d = nc.dram_tensor("x_in", [IN_ROWS, NIMG * XW], f32, kind="ExternalInput").ap()
    c_d = nc.dram_tensor("const_in", [128, CONST_COLS], f32, kind="ExternalInput").ap()
    res_d = nc.dram_tensor("res_out", [NIMG, RPC, OW], f32, kind="ExternalOutput").ap()
    ker_d = nc.dram_tensor("ker_out", [B, 128, KCOLS], f32, kind="ExternalOutput").ap()

    with ExitStack() as ctx:
        const_sb = ctx.enter_context(
            nc.sbuf_tensor("const_sb", [128, CONST_COLS], f32)
        ).ap()
        xt = ctx.enter_context(nc.sbuf_tensor("xt", [IN_ROWS, NIMG * XW], f32)).ap()
        patbig = ctx.enter_context(
            nc.sbuf_tensor("patbig", [128, 2 * KCOLS], f32)
        ).ap()
        # vertical results, one padded tile per in-flight pair
        vps = [
            ctx.enter_context(nc.sbuf_tensor(f"vp{i}", [128, XW], f32)).ap()
            for i in range(NSLOT)
        ]
        rt = ctx.enter_context(nc.sbuf_tensor("rt", [112, NPAIR * OW], f32)).ap()
        # PSUM pair-slots: two banks per slot ([128, 512] + [128, 260])
        psa = [
            ctx.enter_context(nc.psum_tensor(f"psa{i}", [128, NA], f32)).ap()
            for i in range(NSLOT)
        ]
        # full-bank allocations so no tile straddles a PSUM bank boundary
        psb = [
            ctx.enter_context(nc.psum_tensor(f"psb{i}", [128, NA], f32)).ap()[
                :, 0:NB
            ]
            for i in range(NSLOT)
        ]

        s_const = ctx.enter_context(nc.semaphore("s_const"))
        s_chunk = [
            ctx.enter_context(nc.semaphore(f"s_chunk{k}")) for k in range(4)
        ]
        s_ker = [ctx.enter_context(nc.semaphore(f"s_ker{g}")) for g in range(2)]
        s_res = [ctx.enter_context(nc.semaphore(f"s_res{g}")) for g in range(2)]
        psem = ctx.enter_context(nc.semaphore("psem"))   # PE image progress
        asem = ctx.enter_context(nc.semaphore("asem"))   # ACT copy progress
        vsem = ctx.enter_context(nc.semaphore("vsem"))   # pattern fill done
        fsem = ctx.enter_context(nc.semaphore("fsem"))   # DVE same-engine chain
        gsem = ctx.enter_context(nc.semaphore("gsem"))   # gpsimd memset done
        dsem = ctx.enter_context(nc.semaphore("dsem"))   # DVE pair done

        wvt_sb = const_sb[0:IN_ROWS, 36:CONST_COLS]
        blk = ctx.enter_context(nc.Block())

        @blk.sync
        def _(sync):
            sync.dma_start(const_sb, c_d).then_inc(s_const, 16)
            for k in range(4):
                a, b = CHUNKS[k], CHUNKS[k + 1]
                sync.dma_start(
                    xt[:, a * XW : b * XW], x_d[:, a * XW : b * XW]
                ).then_inc(s_chunk[k], 16)
            sync.wait_ge(vsem, 1)
            for g in range(2):
                sync.dma_start(
                    ker_d[2 * g : 2 * g + 2].rearrange("b p f -> p b f"),
                    patbig.rearrange("p (b f) -> p b f", b=2),
                ).then_inc(s_ker[g], 16)
            sync.wait_ge(dsem, NPAIR)
            # rt rows {0:48, 64:112} hold images {2p, 2p+1}
            for g in range(2):
                sync.dma_start(
                    res_d[g : NIMG : 2].rearrange("p r c -> r p c"),
                    rt[64 * g : 64 * g + RPC, :].rearrange(
                        "r (p c) -> r p c", p=NPAIR
                    ),
                ).then_inc(s_res[g], 16)
            sync.wait_ge(s_ker[0], 16)
            sync.wait_ge(s_ker[1], 16)
            sync.wait_ge(s_res[0], 16)
            sync.wait_ge(s_res[1], 16)

        @blk.gpsimd
        def _(gpsimd):
            # zero the vp tiles once so the garbage rows 48:64 read as finite
            for i in range(NSLOT):
                ins = nc.gpsimd.memset(vps[i], 0.0)
            ins.then_inc(gsem, 1)

        @blk.tensor
        def _(tensor):
            tensor.wait_ge(s_const, 16)
            for img in range(NIMG):
                pair, g = divmod(img, 2)
                slot = pair % NSLOT
                if img in CHUNKS:
                    tensor.wait_ge(s_chunk[CHUNKS.index(img)], 16)
                if pair >= NSLOT and g == 0:
                    # psum slot reuse: ACT copies of pair-NSLOT must be done
                    tensor.wait_ge(asem, 2 * (pair - NSLOT) + 2)
                x0 = img * XW
                ro = 64 * g
                tp = (0, ro)
                nc.tensor.matmul(
                    psa[slot][ro : ro + RPC, :],
                    wvt_sb,
                    xt[:, x0 : x0 + NA],
                    start=True,
                    stop=True,
                    tile_position=tp,
                )
                nc.tensor.matmul(
                    psb[slot][ro : ro + RPC, :],
                    wvt_sb,
                    xt[:, x0 + NA : x0 + XW],
                    start=True,
                    stop=True,
                    tile_position=tp,
                ).then_inc(psem, 1)

        @blk.scalar
        def _(scalar):
            scalar.wait_ge(gsem, 1)
            for img in range(NIMG):
                pair, g = divmod(img, 2)
                slot = pair % NSLOT
                ro = 64 * g
                if pair >= NSLOT and g == 0:
                    # vp slot reuse: DVE chain of pair-NSLOT must be done
                    scalar.wait_ge(dsem, pair - NSLOT + 1)
                scalar.wait_ge(psem, img + 1)
                nc.scalar.copy(
                    vps[slot][ro : ro + RPC, 0:NA], psa[slot][ro : ro + RPC, :]
                )
                nc.scalar.copy(
                    vps[slot][ro : ro + RPC, NA:XW], psb[slot][ro : ro + RPC, :]
                ).then_inc(asem, 1)

        @blk.vector
        def _(vector):
            vector.wait_ge(s_const, 16)
            vector.wait_ge(gsem, 1)
            # doubling fill of the kernel-weights pattern; consecutive DVE ops
            # RAW on patbig, chained via fsem for the race detector
            fcnt = 0
            ins = nc.vector.tensor_copy(patbig[:, 0:36], const_sb[:, 0:36])
            filled = 36
            while filled < 2 * KCOLS:
                n = min(filled, 2 * KCOLS - filled)
                ins.then_inc(fsem, 1)
                fcnt += 1
                vector.wait_ge(fsem, fcnt)
                ins = nc.vector.tensor_copy(
                    patbig[:, filled : filled + n], patbig[:, 0:n]
                )
                filled += n
            ins.then_inc(vsem, 1)

            for pair in range(NPAIR):
                slot = pair % NSLOT
                vp = vps[slot]
                out = rt[0:112, pair * OW : (pair + 1) * OW]
                vector.wait_ge(asem, 2 * pair + 2)
                ins = nc.vector.tensor_scalar_mul(
                    out, vp[0:112, 0:767:2], float(WN32[0])
                )
                for q in range(1, PT):
                    ins.then_inc(fsem, 1)
                    fcnt += 1
                    vector.wait_ge(fsem, fcnt)
                    ins = nc.vector.scalar_tensor_tensor(
                        out,
                        vp[0:112, q : q + 767 : 2],
                        float(WN32[q]),
                        out,
                        op0=mybir.AluOpType.mult,
                        op1=mybir.AluOpType.add,
                    )
                ins.then_inc(dsem, 1)

    _CACHE["nc"] = nc
    return nc


def make_in_maps(x):
    xf = np.ascontiguousarray(np.asarray(x, dtype=np.float32)).reshape(NIMG, H, W)
    cols = np.clip(np.arange(-2, W + 2), 0, W - 1)
    in_maps = []
    for r in range(NCORES):
        rows = np.clip(np.arange(96 * r - 2, 96 * r + 98), 0, H - 1)
        slab = xf[:, rows[:, None], cols[None, :]]        # [12, 100, 772]
        slab = slab.transpose(1, 0, 2).reshape(IN_ROWS, NIMG * XW)
        in_maps.append(
            {
                "x_in": np.ascontiguousarray(slab),
                "const_in": CONST,
            }
        )
    return in_maps


def gather_outputs(results):
    res = np.empty((NIMG, OH, OW), np.float32)
    ker = np.empty((B, 1, OH, OW, 36), np.float32)
    for r, m in enumerate(results):
        res[:, RPC * r : RPC * (r + 1), :] = m["res_out"]
        ker[:, 0, RPC * r : RPC * (r + 1)] = m["ker_out"].reshape(B, RPC, OW, 36)
    return res.reshape(B, C, OH, OW), ker


def kernel(input, out_h, out_w):
    assert int(out_h) == OH and int(out_w) == OW
    x = np.asarray(input, dtype=np.float32)
    assert x.shape == (B, C, H, W)
    nc = build_bass()
    out = run_bass_kernel_spmd(nc, make_in_maps(x), core_ids=list(range(NCORES)))
    return gather_outputs(out.results)
